# revision 34
# baseline (speedup 1.0000x reference)
"""Leaky-integrator linear recurrence kernel for Trainium2.

u_t = TAU * u_{t-1} + x_t along the last (time) axis of x[32, 1024, 2048] f32.

Strategy: data-parallel across 8 NeuronCores (4 batches each). The problem is
memory-bound, so HBM traffic is halved by moving data as 16-bit floats (the
2e-2 tolerance dwarfs the quantization error). The recurrence is computed on
the Tensor engine as a *banded matmul*: since TAU^129 < 2e-6, u_t is (to
float precision) a windowed sum u_t = sum_{s=t-255..t} TAU^(t-s) x_s. In a
host-transposed layout xt[time, rows], each 128-step output block i is

    u[i*128+m, r] = sum_{k} A[k, m] * xt[(i-1)*128+k, r]   (cross-block band)
                  + sum_{k} B[k, m] * xt[i*128+k, r]       (triangular band)

with A[k, m] = TAU^(m+128-k), B[k, m] = TAU^(m-k) for k<=m else 0 — two
accumulating 128x128-stationary matmuls per PSUM chunk (block 0 skips A).

Engine assignment: Sync issues input DMAs, Scalar issues output DMAs (two
HWDGE rings, so input prefetch never head-of-line blocks behind output
drain), Tensor does the matmuls, and the PSUM f32 -> SBUF 16-bit downcast
copies are split between Vector and Scalar (each ~46 us; a single engine
at ~92 us would sit on the critical path).

The walrus build in this container allows at most ONE embedded sync-wait
per engine instruction (two on EventSemaphore); Tile's wait assignment can
attach several. _split_excess_waits() hoists the extras onto standalone
EventSemaphore instructions inserted immediately before, on the same
engine — conservative but correct, since every awaited semaphore's
producer precedes the waiter in the scheduled program order.
"""

import numpy as np
import ml_dtypes

import concourse.bass as bass
import concourse.mybir as mybir
from concourse.bass_utils import run_bass_kernel_spmd
from concourse.tile import TileContext

TAU = 0.9
B, F, T = 32, 1024, 2048
N_CORES = 8
B_PER_CORE = B // N_CORES          # 4
ROWS = B_PER_CORE * F              # 4096 independent recurrences per core
P = 128
N_BLK = T // P                     # 16 time-blocks (slabs) per core
CHUNK = 512                        # PSUM bank width (f32)
N_CHUNK = ROWS // CHUNK            # 8

NP_DT = ml_dtypes.bfloat16
MYBIR_DT = mybir.dt.bfloat16

_nc_cache = None
_coef_cache = None
last_results = None  # BassKernelResults from the most recent run (for test.py)


def _split_excess_waits(nc: bass.Bass) -> None:
    for fn in nc.m.functions:
        for blk in fn.blocks:
            out = []
            changed = False
            for inst in blk.instructions:
                si = inst.sync_info
                waits = list(si.on_wait) if si is not None else []
                cap = 2 if inst.opcode == "EventSemaphore" else 1
                if len(waits) <= cap:
                    out.append(inst)
                    continue
                changed = True
                # On DMAs keep a queue-ordering (DMAHW*) wait embedded so
                # queue-level throttling stays at the queue; otherwise keep
                # the last wait.
                keep_idx = len(waits) - 1
                if inst.opcode == "DMACopy":
                    for k, w in enumerate(waits):
                        if (w.ant_name or "").startswith("DMA"):
                            keep_idx = k
                            break
                rest = [w for j, w in enumerate(waits) if j != keep_idx]
                for j in range(0, len(rest), 2):
                    out.append(
                        mybir.InstEventSemaphore(
                            name=f"{inst.name}-xw{j}",
                            opcode="EventSemaphore",
                            engine=inst.engine,
                            debug=inst.debug,
                            sync_info=mybir.SyncInfo(
                                on_wait=rest[j : j + 2], on_update=[]
                            ),
                        )
                    )
                inst.sync_info = mybir.SyncInfo(
                    on_wait=[waits[keep_idx]], on_update=list(si.on_update)
                )
                out.append(inst)
            if changed:
                blk.instructions = out


def _dedup_ldweights(nc: bass.Bass) -> None:
    """Drop PE weight reloads that reload the already-loaded stationary.

    tile_legalize splits every matmul into InstLdweights + a
    non-self-loading InstMatmult. Matmult does not clobber the PE weight
    array, so consecutive Ldweights with an identical weights AP are
    redundant — all but the first can go (saving ~100 ns of PE time each,
    ~21 us total here). A redundant Ldweights that carries semaphore
    waits/updates is replaced by an EventSemaphore on the same engine so
    the synchronization is preserved; any other PE instruction resets the
    tracked signature (conservative).
    """
    for fn in nc.m.functions:
        for blk in fn.blocks:
            out = []
            changed = False
            last_sig = None
            for inst in blk.instructions:
                if inst.opcode == "Matmult":
                    out.append(inst)
                    continue
                if inst.opcode != "Ldweights":
                    if inst.engine == mybir.EngineType.PE and inst.opcode not in (
                        "EventSemaphore",
                    ):
                        last_sig = None
                    out.append(inst)
                    continue
                a = inst.ins[0]
                sig = (a.memref, a.offset, str(a.ap), str(a.dtype))
                if sig != last_sig:
                    last_sig = sig
                    out.append(inst)
                    continue
                changed = True
                si = inst.sync_info
                waits = list(si.on_wait) if si is not None else []
                upds = list(si.on_update) if si is not None else []
                if waits or upds:
                    for j in range(0, max(len(waits), 1), 2):
                        out.append(
                            mybir.InstEventSemaphore(
                                name=f"{inst.name}-lw{j}",
                                opcode="EventSemaphore",
                                engine=inst.engine,
                                debug=inst.debug,
                                sync_info=mybir.SyncInfo(
                                    on_wait=waits[j : j + 2],
                                    on_update=upds if j == 0 else [],
                                ),
                            )
                        )
            if changed:
                blk.instructions = out


def _coef() -> np.ndarray:
    # [P, 2P] = [A | B] packed side by side (one SBUF tile, one DMA):
    #   A[k, m] = TAU^(m+128-k)                (cross-block band)
    #   B[k, m] = TAU^(m-k) for k <= m else 0  (triangular band)
    k = np.arange(2 * P)[:, None]
    m = np.arange(P)[None, :]
    e = m + P - k
    c = np.where(e >= 0, TAU ** np.maximum(e, 0).astype(np.float64), 0.0)
    return np.ascontiguousarray(
        np.hstack([c[:P], c[P:]]).astype(NP_DT)
    )


def _build() -> bass.Bass:
    nc = bass.Bass()
    xt = nc.dram_tensor("xt", [T, ROWS], MYBIR_DT, kind="ExternalInput")
    coef = nc.dram_tensor("coef", [P, 2 * P], MYBIR_DT, kind="ExternalInput")
    yt = nc.dram_tensor("yt", [T, ROWS], MYBIR_DT, kind="ExternalOutput")

    x_r = xt.rearrange("(i p) r -> i p r", p=P)   # 16 slabs [128, ROWS]
    y_r = yt.rearrange("(i p) r -> i p r", p=P)   # 16 blocks [128, ROWS]

    with TileContext(nc) as tc:
        with (
            tc.tile_pool(name="const", bufs=1) as cpool,
            tc.tile_pool(name="in", bufs=8) as ipool,
            tc.tile_pool(name="out", bufs=4) as opool,
            tc.tile_pool(name="psum", bufs=8, space="PSUM") as ppool,
        ):
            cf = cpool.tile([P, 2 * P], MYBIR_DT)
            # coef first on the input queue: it gates the first LDWEIGHTS,
            # and its 128 tiny descriptors cost slab 0 only ~0.5 us
            nc.sync.dma_start(out=cf[:], in_=coef[:])
            cA = cf[:, 0:P]
            cB = cf[:, P : 2 * P]

            LAST = N_BLK - 1
            slabs = []
            for i in range(N_BLK):
                s = ipool.tile([P, ROWS], MYBIR_DT)
                if i == 0:
                    # quarter-granular: the first matmul gates the whole
                    # PE-critical pipeline, start it ASAP
                    q4 = ROWS // 4
                    for qq in range(4):
                        nc.sync.dma_start(
                            out=s[:, qq * q4 : (qq + 1) * q4],
                            in_=x_r[i][:, qq * q4 : (qq + 1) * q4],
                        )
                elif i == 1 or i == LAST:
                    # halves: earlier A-matmuls for block 1 / earlier final
                    # writes as the read stream ends
                    h = ROWS // 2
                    nc.sync.dma_start(out=s[:, 0:h], in_=x_r[i][:, 0:h])
                    nc.sync.dma_start(out=s[:, h:ROWS], in_=x_r[i][:, h:ROWS])
                else:
                    nc.sync.dma_start(out=s[:], in_=x_r[i])
                slabs.append(s)

                utile = opool.tile([P, ROWS], MYBIR_DT)
                # All-A then all-B so the redundant-LDWEIGHTS dedup pass can
                # collapse each group to one weight load; the 8 chunks exactly
                # fill the 8 PSUM banks. Chunk direction alternates per block
                # so block i+1's A-matmuls only become ready (PSUM bank freed)
                # after block i's B-phase — keeping same-weight runs
                # contiguous in the scheduled PE order.
                order = list(range(N_CHUNK))
                if i % 2:
                    order.reverse()
                # B first (start=True resets the full bank), then the A-band
                # accumulated onto rows 0..63 only: A's coefficients for
                # output rows m>=64 are <= TAU^65 ~ 1e-3 — negligible — and
                # a 64-column stationary halves each A LDWEIGHTS (cost
                # scales with stationary columns).
                pts = {}
                for c in order:
                    pt = ppool.tile([P, CHUNK], mybir.dt.float32)
                    pts[c] = pt
                    sl = slice(c * CHUNK, (c + 1) * CHUNK)
                    nc.tensor.matmul(
                        pt[:], lhsT=cB[:], rhs=slabs[i][:, sl],
                        start=True, stop=(i == 0),
                    )
                for c in order:
                    sl = slice(c * CHUNK, (c + 1) * CHUNK)
                    if i > 0:
                        nc.tensor.matmul(
                            pts[c][0:64, :], lhsT=cA[:, 0:64],
                            rhs=slabs[i - 1][:, sl],
                            start=False, stop=True,
                        )
                    if c % 2 == 0:
                        nc.vector.tensor_copy(utile[:, sl], pts[c][:])
                    else:
                        nc.scalar.copy(utile[:, sl], pts[c][:])
                    if i == LAST:
                        # final block streams output per chunk so each write
                        # is ready the moment its copy lands
                        nc.scalar.dma_start(out=y_r[i][:, sl], in_=utile[:, sl])
                if i != LAST:
                    nc.scalar.dma_start(out=y_r[i], in_=utile[:])
                if i >= 1:
                    slabs[i - 1] = None

    _dedup_ldweights(nc)
    _split_excess_waits(nc)
    return nc


def kernel(x: np.ndarray, **_unused) -> np.ndarray:
    global _nc_cache, _coef_cache, last_results
    if _nc_cache is None:
        _nc_cache = _build()
        _coef_cache = _coef()
    nc = _nc_cache

    x = np.asarray(x)
    assert x.shape == (B, F, T), x.shape
    x16 = np.ascontiguousarray(x.reshape(N_CORES, ROWS, T), dtype=NP_DT)
    in_maps = [
        {"xt": np.ascontiguousarray(x16[c].T), "coef": _coef_cache}
        for c in range(N_CORES)
    ]
    last_results = run_bass_kernel_spmd(
        nc, in_maps, core_ids=list(range(N_CORES))
    )
    out = np.concatenate(
        [
            r["yt"].T.astype(np.float32).reshape(B_PER_CORE, F, T)
            for r in last_results.results
        ],
        axis=0,
    )
    return out


# revision 36
# speedup vs baseline: 1.3077x; 1.3077x over previous
"""Leaky-integrator linear recurrence kernel for Trainium2.

u_t = TAU * u_{t-1} + x_t along the last (time) axis of x[32, 1024, 2048] f32.

Strategy: data-parallel across 8 NeuronCores (4 batches each). The problem is
memory-bound, so HBM traffic is halved by moving data as 16-bit floats (the
2e-2 tolerance dwarfs the quantization error). The recurrence is computed on
the Tensor engine as a *banded matmul*: since TAU^129 < 2e-6, u_t is (to
float precision) a windowed sum u_t = sum_{s=t-255..t} TAU^(t-s) x_s. In a
host-transposed layout xt[time, rows], each 128-step output block i is

    u[i*128+m, r] = sum_{k} A[k, m] * xt[(i-1)*128+k, r]   (cross-block band)
                  + sum_{k} B[k, m] * xt[i*128+k, r]       (triangular band)

with A[k, m] = TAU^(m+128-k), B[k, m] = TAU^(m-k) for k<=m else 0 — two
accumulating 128x128-stationary matmuls per PSUM chunk (block 0 skips A).

Engine assignment: Sync issues input DMAs, Scalar issues output DMAs (two
HWDGE rings, so input prefetch never head-of-line blocks behind output
drain), Tensor does the matmuls, and the PSUM f32 -> SBUF 16-bit downcast
copies are split between Vector and Scalar (each ~46 us; a single engine
at ~92 us would sit on the critical path).

The walrus build in this container allows at most ONE embedded sync-wait
per engine instruction (two on EventSemaphore); Tile's wait assignment can
attach several. _split_excess_waits() hoists the extras onto standalone
EventSemaphore instructions inserted immediately before, on the same
engine — conservative but correct, since every awaited semaphore's
producer precedes the waiter in the scheduled program order.
"""

import numpy as np
import ml_dtypes

import concourse.bass as bass
import concourse.mybir as mybir
from concourse.bass_utils import run_bass_kernel_spmd
from concourse.tile import TileContext

TAU = 0.9
B, F, T = 32, 1024, 2048
N_CORES = 8
B_PER_CORE = B // N_CORES          # 4
ROWS = B_PER_CORE * F              # 4096 independent recurrences per core
P = 128
N_BLK = T // P                     # 16 time-blocks (slabs) per core
CHUNK = 512                        # PSUM bank width (f32)
N_CHUNK = ROWS // CHUNK            # 8

NP_DT = ml_dtypes.bfloat16
MYBIR_DT = mybir.dt.bfloat16

_nc_cache = None
_coef_cache = None
last_results = None  # BassKernelResults from the most recent run (for test.py)


def _split_excess_waits(nc: bass.Bass) -> None:
    for fn in nc.m.functions:
        for blk in fn.blocks:
            out = []
            changed = False
            for inst in blk.instructions:
                si = inst.sync_info
                waits = list(si.on_wait) if si is not None else []
                cap = 2 if inst.opcode == "EventSemaphore" else 1
                if len(waits) <= cap:
                    out.append(inst)
                    continue
                changed = True
                # On DMAs keep a queue-ordering (DMAHW*) wait embedded so
                # queue-level throttling stays at the queue; otherwise keep
                # the last wait.
                keep_idx = len(waits) - 1
                if inst.opcode == "DMACopy":
                    for k, w in enumerate(waits):
                        if (w.ant_name or "").startswith("DMA"):
                            keep_idx = k
                            break
                rest = [w for j, w in enumerate(waits) if j != keep_idx]
                for j in range(0, len(rest), 2):
                    out.append(
                        mybir.InstEventSemaphore(
                            name=f"{inst.name}-xw{j}",
                            opcode="EventSemaphore",
                            engine=inst.engine,
                            debug=inst.debug,
                            sync_info=mybir.SyncInfo(
                                on_wait=rest[j : j + 2], on_update=[]
                            ),
                        )
                    )
                inst.sync_info = mybir.SyncInfo(
                    on_wait=[waits[keep_idx]], on_update=list(si.on_update)
                )
                out.append(inst)
            if changed:
                blk.instructions = out


def _dedup_ldweights(nc: bass.Bass) -> None:
    """Drop PE weight reloads that reload the already-loaded stationary.

    tile_legalize splits every matmul into InstLdweights + a
    non-self-loading InstMatmult. Matmult does not clobber the PE weight
    array, so consecutive Ldweights with an identical weights AP are
    redundant — all but the first can go (saving ~100 ns of PE time each,
    ~21 us total here). A redundant Ldweights that carries semaphore
    waits/updates is replaced by an EventSemaphore on the same engine so
    the synchronization is preserved; any other PE instruction resets the
    tracked signature (conservative).
    """
    for fn in nc.m.functions:
        for blk in fn.blocks:
            out = []
            changed = False
            last_sig = None
            for inst in blk.instructions:
                if inst.opcode == "Matmult":
                    out.append(inst)
                    continue
                if inst.opcode != "Ldweights":
                    if inst.engine == mybir.EngineType.PE and inst.opcode not in (
                        "EventSemaphore",
                    ):
                        last_sig = None
                    out.append(inst)
                    continue
                a = inst.ins[0]
                sig = (a.memref, a.offset, str(a.ap), str(a.dtype))
                if sig != last_sig:
                    last_sig = sig
                    out.append(inst)
                    continue
                changed = True
                si = inst.sync_info
                waits = list(si.on_wait) if si is not None else []
                upds = list(si.on_update) if si is not None else []
                if waits or upds:
                    for j in range(0, max(len(waits), 1), 2):
                        out.append(
                            mybir.InstEventSemaphore(
                                name=f"{inst.name}-lw{j}",
                                opcode="EventSemaphore",
                                engine=inst.engine,
                                debug=inst.debug,
                                sync_info=mybir.SyncInfo(
                                    on_wait=waits[j : j + 2],
                                    on_update=upds if j == 0 else [],
                                ),
                            )
                        )
            if changed:
                blk.instructions = out


def _coef() -> np.ndarray:
    # [P, 2P] = [A | B] packed side by side (one SBUF tile, one DMA):
    #   A[k, m] = TAU^(m+128-k)                (cross-block band)
    #   B[k, m] = TAU^(m-k) for k <= m else 0  (triangular band)
    k = np.arange(2 * P)[:, None]
    m = np.arange(P)[None, :]
    e = m + P - k
    c = np.where(e >= 0, TAU ** np.maximum(e, 0).astype(np.float64), 0.0)
    return np.ascontiguousarray(
        np.hstack([c[:P], c[P:]]).astype(NP_DT)
    )


def _build() -> bass.Bass:
    nc = bass.Bass()
    xt = nc.dram_tensor("xt", [T, ROWS], MYBIR_DT, kind="ExternalInput")
    coef = nc.dram_tensor("coef", [P, 2 * P], MYBIR_DT, kind="ExternalInput")
    yt = nc.dram_tensor("yt", [T, ROWS], MYBIR_DT, kind="ExternalOutput")

    x_r = xt.rearrange("(i p) r -> i p r", p=P)   # 16 slabs [128, ROWS]
    y_r = yt.rearrange("(i p) r -> i p r", p=P)   # 16 blocks [128, ROWS]

    with TileContext(nc) as tc:
        with (
            tc.tile_pool(name="const", bufs=1) as cpool,
            tc.tile_pool(name="in", bufs=8) as ipool,
            tc.tile_pool(name="out", bufs=4) as opool,
            tc.tile_pool(name="psum", bufs=8, space="PSUM") as ppool,
        ):
            cf = cpool.tile([P, 2 * P], MYBIR_DT)
            # coef rides the (early-idle) output queue so slab 0's
            # descriptors lead the input ring
            nc.scalar.dma_start(out=cf[:], in_=coef[:])
            cA = cf[:, 0:P]
            cB = cf[:, P : 2 * P]

            LAST = N_BLK - 1
            slabs = []
            for i in range(N_BLK):
                s = ipool.tile([P, ROWS], MYBIR_DT)
                if i <= 1 or i == LAST:
                    # First two slabs: half-granular input so the first
                    # matmuls start ~2 us earlier (PE is the critical path
                    # end to end — the whole pipeline shifts left).
                    # Final slab: likewise, so its writes are ready as the
                    # read stream ends (shortens the exposed tail chain).
                    h = ROWS // 2
                    nc.sync.dma_start(out=s[:, 0:h], in_=x_r[i][:, 0:h])
                    nc.sync.dma_start(out=s[:, h:ROWS], in_=x_r[i][:, h:ROWS])
                else:
                    nc.sync.dma_start(out=s[:], in_=x_r[i])
                slabs.append(s)

                utile = opool.tile([P, ROWS], MYBIR_DT)
                # All-A then all-B so the redundant-LDWEIGHTS dedup pass can
                # collapse each group to one weight load; the 8 chunks exactly
                # fill the 8 PSUM banks. Chunk direction alternates per block
                # so block i+1's A-matmuls only become ready (PSUM bank freed)
                # after block i's B-phase — keeping same-weight runs
                # contiguous in the scheduled PE order.
                order = list(range(N_CHUNK))
                if i % 2:
                    order.reverse()
                pts = {}
                for c in order:
                    pt = ppool.tile([P, CHUNK], mybir.dt.float32)
                    pts[c] = pt
                    sl = slice(c * CHUNK, (c + 1) * CHUNK)
                    if i > 0:
                        nc.tensor.matmul(
                            pt[:], lhsT=cA[:], rhs=slabs[i - 1][:, sl],
                            start=True, stop=False,
                        )
                for c in order:
                    sl = slice(c * CHUNK, (c + 1) * CHUNK)
                    nc.tensor.matmul(
                        pts[c][:], lhsT=cB[:], rhs=slabs[i][:, sl],
                        start=(i == 0), stop=True,
                    )
                    if c % 2 == 0:
                        nc.vector.tensor_copy(utile[:, sl], pts[c][:])
                    else:
                        nc.scalar.copy(utile[:, sl], pts[c][:])
                    if i == LAST:
                        # final block streams output per chunk so each write
                        # is ready the moment its copy lands
                        nc.scalar.dma_start(out=y_r[i][:, sl], in_=utile[:, sl])
                if i != LAST:
                    nc.scalar.dma_start(out=y_r[i], in_=utile[:])
                if i >= 1:
                    slabs[i - 1] = None

    _dedup_ldweights(nc)
    _split_excess_waits(nc)
    return nc


def kernel(x: np.ndarray, **_unused) -> np.ndarray:
    global _nc_cache, _coef_cache, last_results
    if _nc_cache is None:
        _nc_cache = _build()
        _coef_cache = _coef()
    nc = _nc_cache

    x = np.asarray(x)
    assert x.shape == (B, F, T), x.shape
    x16 = np.ascontiguousarray(x.reshape(N_CORES, ROWS, T), dtype=NP_DT)
    in_maps = [
        {"xt": np.ascontiguousarray(x16[c].T), "coef": _coef_cache}
        for c in range(N_CORES)
    ]
    last_results = run_bass_kernel_spmd(
        nc, in_maps, core_ids=list(range(N_CORES))
    )
    out = np.concatenate(
        [
            r["yt"].T.astype(np.float32).reshape(B_PER_CORE, F, T)
            for r in last_results.results
        ],
        axis=0,
    )
    return out
